# revision 7
# baseline (speedup 1.0000x reference)
"""DigitCaps u_hat kernel for Trainium2 (8 NeuronCores, SPMD).

Computes u_hat[b,r,c,o] = sum_i W[0,r,c,o,i] * x[b,r,i] + bias[o,0]
with B=512, R=1152, C=10, O=16, I=8 -> output [512, 1152, 10, 16, 1] f32.

Strategy
--------
Shard R (capsule-route dim) across the 8 cores: 144 r-values per core.
Each core computes its full [B=512, 144, 160] output slice (CO = C*O = 160).

The contraction dim is tiny (I=8), so we pack G=3 r-values per matmul to
keep the moving free dim >= 256 (full-rate for 2-byte dtypes):

  k = (r', i)  with i in [0, 9)   (8 x-values + 1 constant 1.0 for the bias)
  lhsT [128, 128] = x^T for a 128-wide b-block  (stationary)
  rhs  [128, 480] = block-diag W (3 blocks of [9, 160], bias row included)
  out  [128, 480] = psum[b, (r', co)]

The psum tile is [b, (r,co)]-major, so after a cast-copy to SBUF it DMAs to
the [512, 144, 160] HBM output with fully contiguous 128-partition writes.

The kernel is HBM-bandwidth-bound: the f16 output stream (23.6 MB/core)
runs gapless at the 368 GB/s line rate in steady state, so all tuning is
about starting that stream early.  The contraction is zero-padded to K=128
(K<=32 matmuls stream ~1.75x slower, measured), realized as:

  * x chunks are stacked 4-per-tile: supertile [128, cols] rows
    [32q:32q+32) hold chunk 4c+q (27 real contraction rows + 5 DRAM zero
    rows).  No SBUF zeroing for x at all, and the load is a single
    full-width 128-partition DMA that runs at line rate.
  * each W chunk gets its own [128, cols] tile with data in its 32-row
    stratum and 32-row GpSimd memsets zeroing the other three strata
    (engine APs may span >32 partitions only from partition 0).  Matmuls
    for chunk q then contract other chunks' real x rows against exact W
    zeros.  The memsets are partition-disjoint from the W DMA, so nothing
    blocks the input stream (v1 serialized memset->DMA->matmul and idled
    the output ring for 28 us).

A short burst of dummy matmuls on a zeroed scratch tile runs while the
inputs land: the PE clock boost (1.2 -> 2.4 GHz) needs ~3.4 us of
uninterrupted activity, and v1 spent its first 20+ us at half clock.
"""

import numpy as np

# Problem constants (hardcoded per harness contract).
B, R, C, O, I = 512, 1152, 10, 16, 8
CO = C * O                      # 160
NCORES = 8
RS = R // NCORES                # 144 r per core
G = 3                           # r-values packed per matmul
K = G * (I + 1)                 # 27 contraction rows (incl. bias row)
KDMA = 32                       # DRAM rows per chunk (27 real + 5 zero pad)
KPAD = 128                      # full contraction incl. zero strata
N = G * CO                      # 480 moving free dim
NG = RS // G                    # 48 groups per core
CHUNKS = 8                      # input tensors split for early compute start
XSUP = 4                        # x chunks stacked per [128, .] supertile
SLOTS = NG // CHUNKS            # 6 groups per chunk
BBLK = 4                        # 512 / 128 b-blocks
PSUM_GP = 2                     # groups per psum tile (=> 2 banks, 4 tiles)
DMA_GB = 4                      # groups per output DMA (~0.5 MB transfers)
NDUMMY = 8                      # PE warm-up matmuls (512 cols each, ~3.8 us)

OP_DT = "f16"                   # matmul operand dtype: "f32r" | "f16" | "bf16"
OUT_DT = "f16"                  # device output dtype:  "f32"  | "f16"

_prog_cache = {}


def _dt(name):
    from concourse import mybir

    return {
        "f32": mybir.dt.float32,
        "f32r": mybir.dt.float32r,
        "f16": mybir.dt.float16,
        "bf16": mybir.dt.bfloat16,
    }[name]


def _np_dt(name):
    import ml_dtypes

    return {
        "f32": np.float32,
        "f32r": np.float32,
        "f16": np.float16,
        "bf16": ml_dtypes.bfloat16,
    }[name]


def _build_program(op_dt=OP_DT, out_dt=OUT_DT):
    import concourse.bacc as bacc
    import concourse.tile as tile
    from concourse import mybir

    key = (op_dt, out_dt)
    if key in _prog_cache:
        return _prog_cache[key]

    f32 = mybir.dt.float32
    u32 = mybir.dt.uint32
    opd = _dt(op_dt)
    outd = _dt(out_dt)
    esz = mybir.dt.size(opd)
    # Operand tiles/DRAM are uint32-packed: memsets run in 4-byte elements
    # (half the cycles for 2-byte operands) and the bitcast at the matmul is
    # free.  xcol/wcol are the packed column counts.
    xcol = SLOTS * B * esz // 4
    wcol = SLOTS * N * esz // 4

    # Bacc (not raw Bass): its finalize() runs move_matmul_waits_to_ldweights
    # + generate_event_semaphores, required to satisfy the per-instruction
    # sync-wait limits at codegen.
    nc = bacc.Bacc("TRN2", target_bir_lowering=False, debug=False)

    xT_d = nc.declare_dram_parameter(
        "xT", [CHUNKS // XSUP, XSUP * KDMA, xcol], u32, isOutput=False
    )
    Wb_d = nc.declare_dram_parameter("Wb", [CHUNKS, KDMA, wcol], u32, isOutput=False)
    out_d = nc.declare_dram_parameter("out", [B, RS, CO], outd, isOutput=True)

    with tile.TileContext(nc) as tc:
        with (
            tc.tile_pool(name="const", bufs=1) as const,
            tc.tile_pool(name="psum", bufs=8 // PSUM_GP, space="PSUM") as psum,
            tc.tile_pool(name="outp", bufs=8) as outp,
        ):
            # PE warm-up: ~3.8 us of back-to-back dummy matmuls on a zeroed
            # scratch tile, issued while the inputs stream in.  The results
            # land in a psum pool buffer that real matmuls only reach after
            # the dummies retire.
            warm = const.tile([128, 256], u32, tag="warm")
            nc.gpsimd.memset(warm[:], 0)
            ps = psum.tile([128, PSUM_GP, 512], f32, tag="ps")
            for _ in range(NDUMMY):
                nc.tensor.matmul(
                    ps[:, 0, :],
                    warm[:, 0:64].bitcast(opd),
                    warm[:].bitcast(opd),
                    start=True,
                    stop=True,
                )

            # x supertiles: 4 chunks stacked, all-real rows, one line-rate
            # load each on the sync ring (idle until the first output DMA).
            xsb = []
            for sp in range(CHUNKS // XSUP):
                xt = const.tile([XSUP * KDMA, xcol], u32, tag=f"xsb{sp}")
                nc.sync.dma_start(out=xt[:], in_=xT_d[sp])
                xsb.append(xt)

            # W tiles: data in stratum [32q:32q+32), rest zeroed.  Loads on
            # the Act ring (first four up front; the rest issue on the sync
            # ring so they never delay the first psum copies on Act).
            wsb = []
            for ch in range(CHUNKS):
                wt = const.tile([KPAD, wcol], u32, tag=f"wsb{ch}")
                wsb.append(wt)
            for ch in range(CHUNKS):
                q = ch % XSUP
                eng = nc.scalar if ch < 4 else nc.sync
                eng.dma_start(
                    out=wsb[ch][q * KDMA : (q + 1) * KDMA, :], in_=Wb_d[ch]
                )
            for ch in range(CHUNKS):
                q = ch % XSUP
                for z in range(XSUP):
                    if z != q:
                        nc.gpsimd.memset(
                            wsb[ch][z * KDMA : (z + 1) * KDMA, :], 0
                        )

            for j in range(BBLK):
                for qb in range(NG // DMA_GB):
                    ot = outp.tile([128, DMA_GB, N], outd)
                    for t in range(DMA_GB // PSUM_GP):
                        ps = psum.tile([128, PSUM_GP, 512], f32, tag="ps")
                        for u in range(PSUM_GP):
                            g = qb * DMA_GB + t * PSUM_GP + u
                            ch, s = divmod(g, SLOTS)
                            x0 = (s * B + j * 128) * esz // 4
                            x1 = (s * B + (j + 1) * 128) * esz // 4
                            w0 = s * N * esz // 4
                            w1 = (s + 1) * N * esz // 4
                            lhsT = xsb[ch // XSUP][:, x0:x1].bitcast(opd)
                            rhs = wsb[ch][:, w0:w1].bitcast(opd)
                            nc.tensor.matmul(
                                ps[:, u, 0:N],
                                lhsT,
                                rhs,
                                start=True,
                                stop=True,
                            )
                        # Alternate whole-tile copies between the two engines:
                        # amortizes per-instruction overhead; tile-level
                        # latency is hidden by the 4 psum tiles in flight.
                        o0 = t * PSUM_GP
                        if t % 2 == 0:
                            nc.vector.tensor_copy(
                                ot[:, o0 : o0 + PSUM_GP, :], ps[:, :, 0:N]
                            )
                        else:
                            nc.scalar.copy(
                                ot[:, o0 : o0 + PSUM_GP, :], ps[:, :, 0:N]
                            )
                    nc.sync.dma_start(
                        out=out_d[j * 128 : (j + 1) * 128,
                                  qb * DMA_GB * G : (qb + 1) * DMA_GB * G, :],
                        in_=ot[:],
                    )

    nc.finalize()
    _prog_cache[key] = nc
    return nc


def _prep_inputs(x, W, bias, op_dt=OP_DT):
    """Build per-core (xT, Wb) arrays in the device layout."""
    npdt = _np_dt(op_dt)
    x = np.ascontiguousarray(x, dtype=np.float32)
    W = np.ascontiguousarray(W, dtype=np.float32)
    bias = np.ascontiguousarray(bias, dtype=np.float32)

    xx = np.ascontiguousarray(x.transpose(1, 2, 0))      # [R, I, B]
    Wf = W[0].reshape(R, CO, I)                          # [R, CO, I]
    bias_co = np.tile(bias[:, 0], C)                     # [CO]

    in_maps = []
    for c in range(NCORES):
        seg = xx[c * RS : (c + 1) * RS]                  # [RS, I, B]
        seg9 = np.empty((RS, I + 1, B), dtype=npdt)
        seg9[:, :I, :] = seg
        seg9[:, I, :] = 1.0
        # [chunk, slot, r', 9, b] -> [chunk, r'*9+i, slot, b], rows padded
        # to KDMA=32 with zeros, then 4 chunks stacked per supertile row.
        t = seg9.reshape(CHUNKS, SLOTS, G, I + 1, B)
        xT_c = np.zeros((CHUNKS, KDMA, SLOTS * B), dtype=npdt)
        xT_c[:, :K, :] = np.ascontiguousarray(
            t.transpose(0, 2, 3, 1, 4)
        ).reshape(CHUNKS, K, SLOTS * B)
        xT_c = xT_c.reshape(CHUNKS // XSUP, XSUP * KDMA, SLOTS * B)

        Wc = Wf[c * RS : (c + 1) * RS]                   # [RS, CO, I]
        W9 = np.empty((RS, I + 1, CO), dtype=npdt)
        W9[:, :I, :] = Wc.transpose(0, 2, 1)
        W9[:, I, :] = bias_co
        blk = np.zeros((NG, G, I + 1, G, CO), dtype=npdt)
        W9g = W9.reshape(NG, G, I + 1, CO)
        for rp in range(G):
            blk[:, rp, :, rp, :] = W9g[:, rp]
        Wb_c = np.zeros((CHUNKS, KDMA, SLOTS * N), dtype=npdt)
        Wb_c[:, :K, :] = np.ascontiguousarray(
            blk.reshape(CHUNKS, SLOTS, K, N).transpose(0, 2, 1, 3)
        ).reshape(CHUNKS, K, SLOTS * N)

        in_maps.append({"xT": xT_c.view(np.uint32), "Wb": Wb_c.view(np.uint32)})
    return in_maps


def _run(inputs, trace=False, op_dt=OP_DT, out_dt=OUT_DT, **kw):
    from concourse.bass_utils import run_bass_kernel_spmd

    nc = _build_program(op_dt, out_dt)
    in_maps = _prep_inputs(inputs["x"], inputs["W"], inputs["bias"], op_dt)
    res = run_bass_kernel_spmd(
        nc, in_maps, list(range(NCORES)), trace=trace, **kw
    )
    outs = [np.asarray(res.results[c]["out"]) for c in range(NCORES)]
    full = np.concatenate(outs, axis=1)                  # [B, R, CO]
    full = full.astype(np.float32, copy=False)
    return np.ascontiguousarray(full).reshape(B, R, C, O, 1), res


def kernel(x, W, bias):
    out, _ = _run({"x": x, "W": W, "bias": bias})
    return out


# revision 8
# speedup vs baseline: 1.2231x; 1.2231x over previous
"""DigitCaps u_hat kernel for Trainium2 (8 NeuronCores, SPMD).

Computes u_hat[b,r,c,o] = sum_i W[0,r,c,o,i] * x[b,r,i] + bias[o,0]
with B=512, R=1152, C=10, O=16, I=8 -> output [512, 1152, 10, 16, 1] f32.

Strategy
--------
Shard R (capsule-route dim) across the 8 cores: 144 r-values per core.
Each core computes its full [B=512, 144, 160] output slice (CO = C*O = 160).

The contraction dim is tiny (I=8), so we pack G=3 r-values per matmul to
keep the moving free dim >= 256 (full-rate for 2-byte dtypes):

  k = (r', i)  with i in [0, 9)   (8 x-values + 1 constant 1.0 for the bias)
  lhsT [128, 128] = x^T for a 128-wide b-block  (stationary)
  rhs  [128, 480] = block-diag W (3 blocks of [9, 160], bias row included)
  out  [128, 480] = psum[b, (r', co)]

The psum tile is [b, (r,co)]-major, so after a cast-copy to SBUF it DMAs to
the [512, 144, 160] HBM output with fully contiguous 128-partition writes.

The kernel is HBM-bandwidth-bound: the f16 output stream (23.6 MB/core)
runs gapless at the 368 GB/s line rate in steady state, so all tuning is
about starting that stream early.  The contraction is zero-padded to K=128
(K<=32 matmuls stream ~1.75x slower, measured), realized as:

  * x chunks are stacked 4-per-tile: supertile [128, cols] rows
    [32q:32q+32) hold chunk 4c+q (27 real contraction rows + 5 DRAM zero
    rows).  No SBUF zeroing for x at all, and the load is a single
    full-width 128-partition DMA that runs at line rate.
  * each W chunk gets its own [128, cols] tile with data in its 32-row
    stratum and 32-row GpSimd memsets zeroing the other three strata
    (engine APs may span >32 partitions only from partition 0).  Matmuls
    for chunk q then contract other chunks' real x rows against exact W
    zeros.  The memsets are partition-disjoint from the W DMA, so nothing
    blocks the input stream (v1 serialized memset->DMA->matmul and idled
    the output ring for 28 us).

A short burst of dummy matmuls on a zeroed scratch tile runs while the
inputs land: the PE clock boost (1.2 -> 2.4 GHz) needs ~3.4 us of
uninterrupted activity, and v1 spent its first 20+ us at half clock.
"""

import numpy as np

# Problem constants (hardcoded per harness contract).
B, R, C, O, I = 512, 1152, 10, 16, 8
CO = C * O                      # 160
NCORES = 8
RS = R // NCORES                # 144 r per core
G = 3                           # r-values packed per matmul
K = G * (I + 1)                 # 27 contraction rows (incl. bias row)
KDMA = 32                       # DRAM rows per chunk (27 real + 5 zero pad)
KPAD = 128                      # full contraction incl. zero strata
N = G * CO                      # 480 moving free dim
NG = RS // G                    # 48 groups per core
CHUNKS = 8                      # input tensors split for early compute start
XSUP = 4                        # x chunks stacked per [128, .] supertile
SLOTS = NG // CHUNKS            # 6 groups per chunk
BBLK = 4                        # 512 / 128 b-blocks
PSUM_GP = 2                     # groups per psum tile (=> 2 banks, 4 tiles)
DMA_GB = 4                      # groups per output DMA (~0.5 MB transfers)
NDUMMY = 8                      # PE warm-up matmuls (512 cols each, ~3.8 us)

OP_DT = "f16"                   # matmul operand dtype: "f32r" | "f16" | "bf16"
OUT_DT = "f16"                  # device output dtype:  "f32"  | "f16"

_prog_cache = {}


def _dt(name):
    from concourse import mybir

    return {
        "f32": mybir.dt.float32,
        "f32r": mybir.dt.float32r,
        "f16": mybir.dt.float16,
        "bf16": mybir.dt.bfloat16,
    }[name]


def _np_dt(name):
    import ml_dtypes

    return {
        "f32": np.float32,
        "f32r": np.float32,
        "f16": np.float16,
        "bf16": ml_dtypes.bfloat16,
    }[name]


def _build_program(op_dt=OP_DT, out_dt=OUT_DT):
    import concourse.bacc as bacc
    import concourse.tile as tile
    from concourse import mybir

    key = (op_dt, out_dt)
    if key in _prog_cache:
        return _prog_cache[key]

    f32 = mybir.dt.float32
    u32 = mybir.dt.uint32
    opd = _dt(op_dt)
    outd = _dt(out_dt)
    esz = mybir.dt.size(opd)
    # Operand tiles/DRAM are uint32-packed: memsets run in 4-byte elements
    # (half the cycles for 2-byte operands) and the bitcast at the matmul is
    # free.  xcol/wcol are the packed column counts.
    xcol = SLOTS * B * esz // 4
    wcol = SLOTS * N * esz // 4

    # Bacc (not raw Bass): its finalize() runs move_matmul_waits_to_ldweights
    # + generate_event_semaphores, required to satisfy the per-instruction
    # sync-wait limits at codegen.
    nc = bacc.Bacc("TRN2", target_bir_lowering=False, debug=False)

    xT_d = nc.declare_dram_parameter(
        "xT", [CHUNKS // XSUP, XSUP * KDMA, xcol], u32, isOutput=False
    )
    Wb_d = nc.declare_dram_parameter("Wb", [CHUNKS, KDMA, wcol], u32, isOutput=False)
    out_d = nc.declare_dram_parameter("out", [B, RS, CO], outd, isOutput=True)

    with tile.TileContext(nc) as tc:
        with (
            tc.tile_pool(name="const", bufs=1) as const,
            tc.tile_pool(name="psum", bufs=8 // PSUM_GP, space="PSUM") as psum,
            tc.tile_pool(name="outp", bufs=8) as outp,
        ):
            # PE warm-up: ~3.8 us of back-to-back dummy matmuls on a zeroed
            # scratch tile, issued while the inputs stream in.  The results
            # land in a psum pool buffer that real matmuls only reach after
            # the dummies retire.
            warm = const.tile([128, 256], u32, tag="warm")
            nc.gpsimd.memset(warm[:], 0)
            ps = psum.tile([128, PSUM_GP, 512], f32, tag="ps")
            for _ in range(NDUMMY):
                nc.tensor.matmul(
                    ps[:, 0, :],
                    warm[:, 0:64].bitcast(opd),
                    warm[:].bitcast(opd),
                    start=True,
                    stop=True,
                )

            # x supertiles: 4 chunks stacked, all-real rows, one line-rate
            # full-width load each.
            xsb = []
            for sp in range(CHUNKS // XSUP):
                xt = const.tile([XSUP * KDMA, xcol], u32, tag=f"xsb{sp}")
                xsb.append(xt)

            wsb = []
            for ch in range(CHUNKS):
                wt = const.tile([KPAD, wcol], u32, tag=f"wsb{ch}")
                wsb.append(wt)

            def load_w(ch, eng):
                q = ch % XSUP
                eng.dma_start(
                    out=wsb[ch][q * KDMA : (q + 1) * KDMA, :], in_=Wb_d[ch]
                )

            # The r-sweep is the OUTER loop below, so chunk c is first
            # touched ~8 us * 1.5c into the output stream: only x0/W0 gate
            # the stream start (sync ring, ahead of the output DMAs); the
            # rest trickle in on the Act ring with tens of us of slack.
            nc.sync.dma_start(out=xsb[0][:], in_=xT_d[0])
            load_w(0, nc.sync)
            for ch in (1, 2, 3):
                load_w(ch, nc.scalar)
            nc.scalar.dma_start(out=xsb[1][:], in_=xT_d[1])
            for ch in (4, 5, 6, 7):
                load_w(ch, nc.scalar)

            # W pad strata zeroed by 32-row memsets (engine APs may span
            # >32 partitions only from partition 0); partition-disjoint
            # from the W loads, so they never gate the input stream.
            for ch in range(CHUNKS):
                q = ch % XSUP
                for z in range(XSUP):
                    if z != q:
                        nc.gpsimd.memset(
                            wsb[ch][z * KDMA : (z + 1) * KDMA, :], 0
                        )

            for qb in range(NG // DMA_GB):
                for j in range(BBLK):
                    ot = outp.tile([128, DMA_GB, N], outd)
                    for t in range(DMA_GB // PSUM_GP):
                        ps = psum.tile([128, PSUM_GP, 512], f32, tag="ps")
                        for u in range(PSUM_GP):
                            g = qb * DMA_GB + t * PSUM_GP + u
                            ch, s = divmod(g, SLOTS)
                            x0 = (s * B + j * 128) * esz // 4
                            x1 = (s * B + (j + 1) * 128) * esz // 4
                            w0 = s * N * esz // 4
                            w1 = (s + 1) * N * esz // 4
                            lhsT = xsb[ch // XSUP][:, x0:x1].bitcast(opd)
                            rhs = wsb[ch][:, w0:w1].bitcast(opd)
                            nc.tensor.matmul(
                                ps[:, u, 0:N],
                                lhsT,
                                rhs,
                                start=True,
                                stop=True,
                            )
                        # Alternate whole-tile copies between the two engines:
                        # amortizes per-instruction overhead; tile-level
                        # latency is hidden by the 4 psum tiles in flight.
                        o0 = t * PSUM_GP
                        if t % 2 == 0:
                            nc.vector.tensor_copy(
                                ot[:, o0 : o0 + PSUM_GP, :], ps[:, :, 0:N]
                            )
                        else:
                            nc.scalar.copy(
                                ot[:, o0 : o0 + PSUM_GP, :], ps[:, :, 0:N]
                            )
                    nc.sync.dma_start(
                        out=out_d[j * 128 : (j + 1) * 128,
                                  qb * DMA_GB * G : (qb + 1) * DMA_GB * G, :],
                        in_=ot[:],
                    )

    nc.finalize()
    _prog_cache[key] = nc
    return nc


def _prep_inputs(x, W, bias, op_dt=OP_DT):
    """Build per-core (xT, Wb) arrays in the device layout."""
    npdt = _np_dt(op_dt)
    x = np.ascontiguousarray(x, dtype=np.float32)
    W = np.ascontiguousarray(W, dtype=np.float32)
    bias = np.ascontiguousarray(bias, dtype=np.float32)

    xx = np.ascontiguousarray(x.transpose(1, 2, 0))      # [R, I, B]
    Wf = W[0].reshape(R, CO, I)                          # [R, CO, I]
    bias_co = np.tile(bias[:, 0], C)                     # [CO]

    in_maps = []
    for c in range(NCORES):
        seg = xx[c * RS : (c + 1) * RS]                  # [RS, I, B]
        seg9 = np.empty((RS, I + 1, B), dtype=npdt)
        seg9[:, :I, :] = seg
        seg9[:, I, :] = 1.0
        # [chunk, slot, r', 9, b] -> [chunk, r'*9+i, slot, b], rows padded
        # to KDMA=32 with zeros, then 4 chunks stacked per supertile row.
        t = seg9.reshape(CHUNKS, SLOTS, G, I + 1, B)
        xT_c = np.zeros((CHUNKS, KDMA, SLOTS * B), dtype=npdt)
        xT_c[:, :K, :] = np.ascontiguousarray(
            t.transpose(0, 2, 3, 1, 4)
        ).reshape(CHUNKS, K, SLOTS * B)
        xT_c = xT_c.reshape(CHUNKS // XSUP, XSUP * KDMA, SLOTS * B)

        Wc = Wf[c * RS : (c + 1) * RS]                   # [RS, CO, I]
        W9 = np.empty((RS, I + 1, CO), dtype=npdt)
        W9[:, :I, :] = Wc.transpose(0, 2, 1)
        W9[:, I, :] = bias_co
        blk = np.zeros((NG, G, I + 1, G, CO), dtype=npdt)
        W9g = W9.reshape(NG, G, I + 1, CO)
        for rp in range(G):
            blk[:, rp, :, rp, :] = W9g[:, rp]
        Wb_c = np.zeros((CHUNKS, KDMA, SLOTS * N), dtype=npdt)
        Wb_c[:, :K, :] = np.ascontiguousarray(
            blk.reshape(CHUNKS, SLOTS, K, N).transpose(0, 2, 1, 3)
        ).reshape(CHUNKS, K, SLOTS * N)

        in_maps.append({"xT": xT_c.view(np.uint32), "Wb": Wb_c.view(np.uint32)})
    return in_maps


def _run(inputs, trace=False, op_dt=OP_DT, out_dt=OUT_DT, **kw):
    from concourse.bass_utils import run_bass_kernel_spmd

    nc = _build_program(op_dt, out_dt)
    in_maps = _prep_inputs(inputs["x"], inputs["W"], inputs["bias"], op_dt)
    res = run_bass_kernel_spmd(
        nc, in_maps, list(range(NCORES)), trace=trace, **kw
    )
    outs = [np.asarray(res.results[c]["out"]) for c in range(NCORES)]
    full = np.concatenate(outs, axis=1)                  # [B, R, CO]
    full = full.astype(np.float32, copy=False)
    return np.ascontiguousarray(full).reshape(B, R, C, O, 1), res


def kernel(x, W, bias):
    out, _ = _run({"x": x, "W": W, "bias": bias})
    return out
